# revision 2
# baseline (speedup 1.0000x reference)
"""Trainium2 Bass kernel for a 2-branch stacked-GAT network (8 NeuronCores).

v2 design (optimized for axon dispatch wall-time):
  - Nodes are partitioned across the 8 cores (LPT by in-degree), then sorted
    by in-degree within each core so each 128-node block has a near-uniform
    max degree K[b].  Edge layout: partition = dst lane, free dim = that
    node's incoming edges (k-slots padded to K[b] with a sentinel src row).
  - Each GAT layer = node GEMM (fp16) -> all-gather of the fp16 feature
    table -> per-block edge softmax done entirely with free-dim reduces on
    the Vector engine (exact per-node max; no selection matrices, no PE work,
    no er gathers).  Messages are weighted in-place and reduced along k.
  - Pad k-slots gather a sentinel table row (feat=0, el=-60000) so their
    softmax weight underflows to 0 and their message contribution is 0.
  - All tables/weights/activations are fp16 (fp32 PSUM + fp32 softmax
    arithmetic); host I/O is fp16 everywhere; indices ship as int16.
  - A persistent jax compilation cache makes repeat dispatches skip XLA
    compilation of the (per-call rebuilt) shard_map closure.
"""

import os
import tempfile

import numpy as np

import sys
sys.path.insert(0, "/opt/trn_rl_repo")

# jax persistent compilation cache: run_bass_kernel_spmd builds a fresh jit
# closure per call; the persistent cache makes every call after the first a
# cache hit instead of a full XLA compile.
import jax

jax.config.update(
    "jax_compilation_cache_dir",
    os.path.join(tempfile.gettempdir(), "jaxcache_gat"),
)
jax.config.update("jax_persistent_cache_min_compile_time_secs", 0.0)
jax.config.update("jax_persistent_cache_min_entry_size_bytes", -1)

# ----------------------------------------------------------------------------
# problem constants (hardcoded per the task contract)
# ----------------------------------------------------------------------------
N_NODES = 50000
N_EDGES = 800000
IN_DIM = 256
C_OUT = 40
N_CORES = 8
PART = 128
BLOCKS = 49                      # 49 * 128 = 6272 node slots per core
SLOTS = BLOCKS * PART            # 6272
S_TOT = N_CORES * SLOTS          # 50176; sentinel row index == S_TOT
EL_SENT = -60000.0               # sentinel el (fp16-safe; exp underflows to 0)
IDX_OFF = 32768                  # int16 shipping offset for src indices

A_ROW, B_ROW, C_ROW = 520, 130, 82
Y_SCALE = 38.0                   # int8 output quantization scale
A_W, B_W, C_W = 528, 138, 90

_COMPILED = {}
_STATE = {}


# ----------------------------------------------------------------------------
# host-side graph scheduling
# ----------------------------------------------------------------------------
def _balanced_assign(weights, n_bins, cap):
    """Greedy LPT: heaviest item to lightest non-full bin. Returns bin ids."""
    import heapq

    order = np.argsort(-weights, kind="stable")
    loads = np.zeros(n_bins, dtype=np.int64)
    counts = np.zeros(n_bins, dtype=np.int64)
    out = np.empty(len(weights), dtype=np.int32)
    heap = [(0, b) for b in range(n_bins)]
    heapq.heapify(heap)
    for i in order:
        spill = []
        while True:
            load, b = heapq.heappop(heap)
            if counts[b] < cap:
                break
            spill.append((load, b))
        out[i] = b
        counts[b] += 1
        loads[b] += weights[i]
        heapq.heappush(heap, (loads[b], b))
        for item in spill:
            heapq.heappush(heap, item)
    return out


def _schedule(src, dst):
    """Shard nodes across cores; sort by degree into blocks; build edge slots."""
    deg = np.bincount(dst, minlength=N_NODES).astype(np.int64)
    node_core = _balanced_assign(deg, N_CORES, SLOTS)

    # slot within core = rank by degree (desc, stable)
    slot_in_core = np.zeros(N_NODES, dtype=np.int64)
    deg_sorted = np.zeros((N_CORES, SLOTS), dtype=np.int64)
    for c in range(N_CORES):
        nodes = np.where(node_core == c)[0]
        order = np.argsort(-deg[nodes], kind="stable")
        slot_in_core[nodes[order]] = np.arange(len(nodes))
        deg_sorted[c, : len(nodes)] = deg[nodes[order]]

    # K[b] = max in-degree in block b across all cores (sorted desc => first)
    K = deg_sorted[:, ::PART].max(axis=0)
    K = np.maximum(K, 1).astype(np.int64)
    off = np.zeros(BLOCKS + 1, dtype=np.int64)
    off[1:] = np.cumsum(K)
    sumk = int(off[-1])

    global_slot = node_core.astype(np.int64) * SLOTS + slot_in_core

    # per-edge placement: lane = slot%128, block = slot//128, k = rank in dst
    e_core = node_core[dst]
    e_slot = slot_in_core[dst]
    e_blk = e_slot // PART
    e_lane = e_slot % PART
    eorder = np.argsort(dst, kind="stable")
    dst_s = dst[eorder]
    grp_start = np.searchsorted(dst_s, np.arange(N_NODES), side="left")
    k_rank = np.arange(N_EDGES, dtype=np.int64) - grp_start[dst_s]
    kk = np.empty(N_EDGES, dtype=np.int64)
    kk[eorder] = k_rank

    sidx16 = np.full((N_CORES, PART, sumk), S_TOT - IDX_OFF, dtype=np.int16)
    col = off[e_blk] + kk
    sidx16[e_core, e_lane, col] = (global_slot[src] - IDX_OFF).astype(np.int16)

    return dict(
        K=tuple(int(k) for k in K),
        off=off,
        sumk=sumk,
        node_core=node_core,
        slot_in_core=slot_in_core,
        sidx16=sidx16,
    )


def _aug_w(W, al, ar):
    H, D = al.shape
    Wl = np.stack([W[:, h * D:(h + 1) * D] @ al[h] for h in range(H)], axis=1)
    Wr = np.stack([W[:, h * D:(h + 1) * D] @ ar[h] for h in range(H)], axis=1)
    return Wl.astype(np.float32), Wr.astype(np.float32)


def _prep_inputs(inputs, sched):
    """Build all per-core (and shared) device input arrays."""
    x = np.asarray(inputs["x"], np.float32)
    p32 = {k: np.asarray(v, np.float32) for k, v in inputs.items()
           if k not in ("x", "src", "dst")}

    # x in slot order, transposed for the GEMM lhsT, fp16
    xs = np.zeros((S_TOT, IN_DIM), np.float32)
    gs = sched["node_core"].astype(np.int64) * SLOTS + sched["slot_in_core"]
    xs[gs] = x
    xT = np.ascontiguousarray(
        xs.T.reshape(2, PART, S_TOT).transpose(1, 0, 2)).astype(np.float16)

    # layer-A weights [W00 256 | Wl0 4 | W10 256 | Wl1 4 | Wr0 4 | Wr1 4]
    Wl0, Wr0 = _aug_w(p32["W00"], p32["a00l"], p32["a00r"])
    Wl1, Wr1 = _aug_w(p32["W10"], p32["a10l"], p32["a10r"])
    WA = np.zeros((IN_DIM, A_W), np.float32)
    WA[:, 0:256] = p32["W00"]
    WA[:, 256:260] = Wl0
    WA[:, 260:516] = p32["W10"]
    WA[:, 516:520] = Wl1
    WA[:, 520:524] = Wr0
    WA[:, 524:528] = Wr1
    WA16 = np.ascontiguousarray(
        WA.reshape(2, PART, A_W).transpose(1, 0, 2)).astype(np.float16)

    # layer-B weights [W01 64 | Wl2 1 | W1f 64 | Wl5 1 | Wr2 1 | Wr5 1 | pad]
    Wl2, Wr2 = _aug_w(p32["W01"], p32["a01l"], p32["a01r"])
    Wl5, Wr5 = _aug_w(p32["W1f"], p32["a1fl"], p32["a1fr"])
    WB = np.zeros((512, B_W), np.float32)
    WB[0:256, 0:64] = p32["W01"]
    WB[0:256, 64:65] = Wl2
    WB[256:512, 65:129] = p32["W1f"]
    WB[256:512, 129:130] = Wl5
    WB[0:256, 130:131] = Wr2
    WB[256:512, 131:132] = Wr5
    WB16 = np.ascontiguousarray(
        WB.reshape(4, PART, B_W).transpose(1, 0, 2)).astype(np.float16)

    # layer-C weights [W0f 40 | Wl3 1 | W1o 40 | Wl6 1 | Wr3 1 | Wr6 1 | pad]
    Wl3, Wr3 = _aug_w(p32["W0f"], p32["a0fl"], p32["a0fr"])
    Wl6, Wr6 = _aug_w(p32["W1o"], p32["a1ol"], p32["a1or"])
    WC = np.zeros((PART, C_W), np.float32)
    WC[0:64, 0:40] = p32["W0f"]
    WC[0:64, 40:41] = Wl3
    WC[64:128, 41:81] = p32["W1o"]
    WC[64:128, 81:82] = Wl6
    WC[0:64, 82:83] = Wr3
    WC[64:128, 83:84] = Wr6
    WC16 = np.ascontiguousarray(
        WC.reshape(1, PART, C_W).transpose(1, 0, 2)).astype(np.float16)

    # pack everything into ONE fp16 blob per core: fewer axon transfers
    # (per-transfer overhead dominates the upload path)
    in_maps = []
    for c in range(N_CORES):
        blob = np.concatenate([
            xT[:, :, c * SLOTS:(c + 1) * SLOTS].reshape(PART, -1),
            WA16.reshape(PART, -1),
            WB16.reshape(PART, -1),
            WC16.reshape(PART, -1),
            sched["sidx16"][c].view(np.float16),
        ], axis=1)
        in_maps.append(dict(blob=np.ascontiguousarray(blob)))
    return in_maps


# ----------------------------------------------------------------------------
# device program
# ----------------------------------------------------------------------------
def _build(K):
    import concourse.bass as bass
    import concourse.tile as tile
    from concourse import bacc, mybir

    f32 = mybir.dt.float32
    f16 = mybir.dt.float16
    i32 = mybir.dt.int32
    i16 = mybir.dt.int16
    i8 = mybir.dt.int8
    ALU = mybir.AluOpType
    ACT = mybir.ActivationFunctionType
    X = mybir.AxisListType.X

    off = np.zeros(BLOCKS + 1, dtype=np.int64)
    off[1:] = np.cumsum(np.asarray(K))
    sumk = int(off[-1])
    kmax = int(max(K))

    nc = bacc.Bacc("TRN2", target_bir_lowering=False, debug=False,
                   num_devices=N_CORES)

    # ---- I/O (single packed fp16 blob per core) ----
    OFF_X = 0
    OFF_WA = OFF_X + 2 * SLOTS
    OFF_WB = OFF_WA + 2 * A_W
    OFF_WC = OFF_WB + 4 * B_W
    OFF_SIDX = OFF_WC + C_W
    BLOB_W = OFF_SIDX + sumk
    blob = nc.dram_tensor("blob", [PART, BLOB_W], f16, kind="ExternalInput")
    xTl16 = blob[:, OFF_X:OFF_WA].rearrange("p (k s) -> p k s", k=2)
    y_d = nc.dram_tensor("y", [SLOTS, 2 * C_OUT], i8, kind="ExternalOutput")

    # ---- internal DRAM ----
    tbA_sh = nc.dram_tensor("tbA_sh", [SLOTS, A_ROW], f16)
    tableA = nc.dram_tensor("tableA", [S_TOT + 1, A_ROW], f16,
                            addr_space="Shared")
    aggA = nc.dram_tensor("aggA", [SLOTS, 512], f16)
    tbB_sh = nc.dram_tensor("tbB_sh", [SLOTS, B_ROW], f16)
    tableB = nc.dram_tensor("tableB", [S_TOT + 1, B_ROW], f16,
                            addr_space="Shared")
    aggB = nc.dram_tensor("aggB", [SLOTS, PART], f16)
    tbC_sh = nc.dram_tensor("tbC_sh", [SLOTS, C_ROW], f16)
    tableC = nc.dram_tensor("tableC", [S_TOT + 1, C_ROW], f16,
                            addr_space="Shared")

    groups = [list(range(N_CORES))]

    with tile.TileContext(nc, trace_sim=False) as tc:
        with tc.tile_pool(name="const", bufs=1) as cpool, \
             tc.tile_pool(name="gemm_in", bufs=3) as gip, \
             tc.tile_pool(name="gemm_out", bufs=3) as gop, \
             tc.tile_pool(name="gath", bufs=2) as gap, \
             tc.tile_pool(name="small", bufs=3) as smp, \
             tc.tile_pool(name="epi", bufs=2) as epp:

            # ---- constants / resident tiles ----
            wa_t = cpool.tile([PART, 2, A_W], f16)
            nc.sync.dma_start(wa_t[:].rearrange("p k w -> p (k w)"),
                              blob[:, OFF_WA:OFF_WB])
            wb_t = cpool.tile([PART, 4, B_W], f16)
            nc.sync.dma_start(wb_t[:].rearrange("p k w -> p (k w)"),
                              blob[:, OFF_WB:OFF_WC])
            wc_t = cpool.tile([PART, 1, C_W], f16)
            nc.sync.dma_start(wc_t[:].rearrange("p k w -> p (k w)"),
                              blob[:, OFF_WC:OFF_SIDX])

            s16 = cpool.tile([PART, sumk], i16)
            nc.sync.dma_start(s16[:], blob[:, OFF_SIDX:BLOB_W].bitcast(i16))
            s32 = cpool.tile([PART, sumk], i32)
            nc.vector.tensor_copy(s32[:], s16[:])
            nc.vector.tensor_scalar(out=s32[:], in0=s32[:], scalar1=IDX_OFF,
                                    scalar2=None, op0=ALU.add)

            erA_sb = cpool.tile([PART, BLOCKS, 8], f16)
            erB_sb = cpool.tile([PART, BLOCKS, 2], f16)
            erC_sb = cpool.tile([PART, BLOCKS, 2], f16)

            # sentinel rows (feat 0, el slots EL_SENT)
            sentA = cpool.tile([1, A_ROW], f16)
            nc.vector.memset(sentA[:], 0.0)
            nc.vector.memset(sentA[:, 256:260], EL_SENT)
            nc.vector.memset(sentA[:, 516:520], EL_SENT)
            nc.sync.dma_start(tableA[S_TOT:S_TOT + 1, :], sentA[:])
            sentB = cpool.tile([1, B_ROW], f16)
            nc.vector.memset(sentB[:], 0.0)
            nc.vector.memset(sentB[:, 64:65], EL_SENT)
            nc.vector.memset(sentB[:, 129:130], EL_SENT)
            nc.sync.dma_start(tableB[S_TOT:S_TOT + 1, :], sentB[:])
            sentC = cpool.tile([1, C_ROW], f16)
            nc.vector.memset(sentC[:], 0.0)
            nc.vector.memset(sentC[:, 40:41], EL_SENT)
            nc.vector.memset(sentC[:, 81:82], EL_SENT)
            nc.sync.dma_start(tableC[S_TOT:S_TOT + 1, :], sentC[:])

            # ---- phase 1: GEMM-A (node-sharded) + all-gather ----
            with tc.tile_pool(name="psA", bufs=2, space="PSUM") as gpp:
                for b in range(BLOCKS):
                    sl = slice(b * PART, (b + 1) * PART)
                    xt = gip.tile([PART, 2, PART], f16, tag="xt")
                    nc.sync.dma_start(xt[:], xTl16[:, :, sl])
                    ps = gpp.tile([PART, A_W], f32, space="PSUM", tag="psA")
                    for k2 in range(2):
                        nc.tensor.matmul(ps[:, 0:512], lhsT=xt[:, k2, :],
                                         rhs=wa_t[:, k2, 0:512],
                                         start=(k2 == 0), stop=(k2 == 1))
                        nc.tensor.matmul(ps[:, 512:A_W], lhsT=xt[:, k2, :],
                                         rhs=wa_t[:, k2, 512:A_W],
                                         start=(k2 == 0), stop=(k2 == 1))
                    row = gop.tile([PART, A_ROW], f16, tag="rowA")
                    if b % 2 == 0:
                        nc.vector.tensor_copy(row[:], ps[:, 0:A_ROW])
                    else:
                        nc.scalar.copy(row[:], ps[:, 0:A_ROW])
                    nc.vector.tensor_copy(erA_sb[:, b, :], ps[:, A_ROW:A_W])
                    nc.sync.dma_start(tbA_sh[sl, :], row[:])
            nc.gpsimd.collective_compute(
                "AllGather", ALU.bypass, replica_groups=groups,
                ins=[tbA_sh[:, :]], outs=[tableA[0:S_TOT, :]])

            # ---- edge phase helper ----
            def edge_phase(table, row_w, er_sb, fdim, nheads, out_cb):
                """One GAT aggregation layer over all blocks (both branches).

                row layout per branch: [feat fdim*nheads | el nheads]
                """
                fw = fdim * nheads
                r = fw + nheads
                for b in range(BLOCKS):
                    kb = K[b]
                    g = gap.tile([PART, kb, row_w], f16, tag="g")
                    for k in range(kb):
                        c0 = int(off[b]) + k
                        nc.gpsimd.indirect_dma_start(
                            out=g[:, k, :], out_offset=None,
                            in_=table[:, :],
                            in_offset=bass.IndirectOffsetOnAxis(
                                ap=s32[:, c0:c0 + 1], axis=0))
                    # e = el + er  [128, kb, 2, H] f32
                    el = g[:].rearrange("p k (b2 r) -> p k b2 r", b2=2)[
                        :, :, :, fw:fw + nheads]
                    e = smp.tile([PART, kb, 2, nheads], f32, tag="e")
                    erv = er_sb[:, b, :].rearrange(
                        "p (o b2 h) -> p o b2 h", o=1, b2=2)
                    nc.vector.tensor_tensor(
                        out=e[:], in0=el,
                        in1=erv.to_broadcast([PART, kb, 2, nheads]),
                        op=ALU.add)
                    # m = lrelu(max_k e); e = lrelu(e) - m
                    # (ACT.Lrelu ignores alpha and uses slope 0.01, so leaky
                    # relu is computed manually: max(x, 0.2*x))
                    m = smp.tile([PART, 1, 2, nheads], f32, tag="m")
                    nc.vector.tensor_reduce(
                        out=m[:], in_=e[:].rearrange("p k b2 h -> p b2 h k"),
                        axis=X, op=ALU.max)
                    m2 = smp.tile([PART, 1, 2, nheads], f32, tag="m2")
                    nc.vector.tensor_scalar(out=m2[:], in0=m[:], scalar1=0.2,
                                            scalar2=None, op0=ALU.mult)
                    nc.vector.tensor_tensor(out=m[:], in0=m[:], in1=m2[:],
                                            op=ALU.max)
                    e2 = smp.tile([PART, kb, 2, nheads], f32, tag="e2")
                    nc.vector.tensor_scalar(out=e2[:], in0=e[:], scalar1=0.2,
                                            scalar2=None, op0=ALU.mult)
                    nc.vector.tensor_tensor(out=e[:], in0=e[:], in1=e2[:],
                                            op=ALU.max)
                    nc.vector.tensor_tensor(
                        out=e[:], in0=e[:],
                        in1=m[:].to_broadcast([PART, kb, 2, nheads]),
                        op=ALU.subtract)
                    ex = smp.tile([PART, kb, 2, nheads], f16, tag="ex")
                    nc.scalar.activation(ex[:], e[:], ACT.Exp)
                    # den / reciprocal
                    den = smp.tile([PART, 1, 2, nheads], f32, tag="den")
                    nc.vector.tensor_reduce(
                        out=den[:], in_=ex[:].rearrange("p k b2 h -> p b2 h k"),
                        axis=X, op=ALU.add)
                    nc.vector.tensor_scalar(out=den[:], in0=den[:],
                                            scalar1=1e-9, scalar2=None,
                                            op0=ALU.max)
                    rec = smp.tile([PART, 1, 2, nheads], f32, tag="rec")
                    nc.vector.reciprocal(rec[:], den[:])
                    # g(feat) *= ex
                    gf = g[:].rearrange("p k (b2 r) -> p k b2 r", b2=2)[
                        :, :, :, 0:fw].rearrange(
                        "p k b2 (h d) -> p k b2 h d", h=nheads)
                    exb = ex[:].rearrange(
                        "p k b2 (h o) -> p k b2 h o", o=1).to_broadcast(
                        [PART, kb, 2, nheads, fdim])
                    nc.vector.tensor_tensor(out=gf, in0=gf, in1=exb,
                                            op=ALU.mult)
                    # msum over k -> [128, 2, H, fdim] f32; rst = msum * rec
                    ms = epp.tile([PART, 2, nheads, fdim], f32, tag="ms")
                    gfk = g[:].rearrange("p k (b2 r) -> p k b2 r", b2=2)[
                        :, :, :, 0:fw].rearrange(
                        "p k b2 (h d) -> p b2 h d k", h=nheads)
                    nc.vector.tensor_reduce(out=ms[:], in_=gfk, axis=X,
                                            op=ALU.add)
                    o = epp.tile([PART, 2 * fw], f32, tag="o")
                    ov = o[:].rearrange("p (b2 h d) -> p b2 h d", b2=2,
                                        h=nheads)
                    recb = rec[:].rearrange(
                        "p o b2 (h o2) -> p o b2 h o2", o2=1)[:, 0]
                    nc.vector.tensor_tensor(
                        out=ov, in0=ms[:],
                        in1=recb.to_broadcast([PART, 2, nheads, fdim]),
                        op=ALU.mult)
                    out_cb(b, o)

            def elu_inplace(ap, width):
                """ap <- elu(ap): relu(x) + exp(min(x,0)) - 1."""
                tm = epp.tile([PART, width], f32, tag="elu_t")
                nc.vector.tensor_scalar(out=tm[:], in0=ap, scalar1=0.0,
                                        scalar2=None, op0=ALU.min)
                te = epp.tile([PART, width], f32, tag="elu_e")
                nc.scalar.activation(te[:], tm[:], ACT.Exp)
                nc.scalar.activation(tm[:], ap, ACT.Relu)
                nc.vector.tensor_tensor(out=te[:], in0=te[:], in1=tm[:],
                                        op=ALU.add)
                nc.vector.tensor_scalar(out=ap, in0=te[:], scalar1=-1.0,
                                        scalar2=None, op0=ALU.add)

            # ---- edge phase A -> aggA ----
            def out_a(b, o):
                elu_inplace(o[:], 512)
                o16 = epp.tile([PART, 512], f16, tag="o16")
                nc.scalar.copy(o16[:], o[:])
                nc.sync.dma_start(aggA[b * PART:(b + 1) * PART, :], o16[:])

            edge_phase(tableA, A_ROW, erA_sb, 64, 4, out_a)

            # ---- phase 3: GEMM-B (sharded) + all-gather ----
            with tc.tile_pool(name="psB", bufs=2, space="PSUM") as gpp:
                for b in range(BLOCKS):
                    sl = slice(b * PART, (b + 1) * PART)
                    hT = gip.tile([PART, 4, PART], f16, tag="hT")
                    for k2 in range(4):
                        nc.sync.dma_start_transpose(
                            hT[:, k2, :],
                            aggA[sl, k2 * PART:(k2 + 1) * PART])
                    psb = gpp.tile([PART, B_W], f32, space="PSUM", tag="psB")
                    for k2 in range(4):
                        nc.tensor.matmul(psb[:], lhsT=hT[:, k2, :],
                                         rhs=wb_t[:, k2, :],
                                         start=(k2 == 0), stop=(k2 == 3))
                    rowb = gop.tile([PART, B_ROW], f16, tag="rowB")
                    nc.vector.tensor_copy(rowb[:], psb[:, 0:B_ROW])
                    nc.vector.tensor_copy(erB_sb[:, b, :], psb[:, 130:132])
                    nc.sync.dma_start(tbB_sh[sl, :], rowb[:])
            nc.gpsimd.collective_compute(
                "AllGather", ALU.bypass, replica_groups=groups,
                ins=[tbB_sh[:, :]], outs=[tableB[0:S_TOT, :]])

            # ---- edge phase B -> aggB ----
            def out_b(b, o):
                elu_inplace(o[:, 0:64], 64)
                o16 = epp.tile([PART, PART], f16, tag="o16b")
                nc.scalar.copy(o16[:], o[:])
                nc.sync.dma_start(aggB[b * PART:(b + 1) * PART, :], o16[:])

            edge_phase(tableB, B_ROW, erB_sb, 64, 1, out_b)

            # ---- phase 5: GEMM-C (sharded) + all-gather ----
            with tc.tile_pool(name="psC", bufs=2, space="PSUM") as gpp:
                for b in range(BLOCKS):
                    sl = slice(b * PART, (b + 1) * PART)
                    hc = gip.tile([PART, PART], f16, tag="hc")
                    nc.sync.dma_start_transpose(hc[:], aggB[sl, :])
                    psc = gpp.tile([PART, C_W], f32, space="PSUM", tag="psC")
                    nc.tensor.matmul(psc[:], lhsT=hc[:], rhs=wc_t[:, 0, :],
                                     start=True, stop=True)
                    rowc = gop.tile([PART, C_ROW], f16, tag="rowC")
                    nc.vector.tensor_copy(rowc[:], psc[:, 0:C_ROW])
                    nc.vector.tensor_copy(erC_sb[:, b, :], psc[:, 82:84])
                    nc.sync.dma_start(tbC_sh[sl, :], rowc[:])
            nc.gpsimd.collective_compute(
                "AllGather", ALU.bypass, replica_groups=groups,
                ins=[tbC_sh[:, :]], outs=[tableC[0:S_TOT, :]])

            # ---- edge phase C -> y ----
            def out_c(b, o):
                elu_inplace(o[:, 40:80], 40)
                # int8 output: y in [-3.37, 3.37], quant step 1/38 (rounds+
                # saturates on conversion); host divides by Y_SCALE
                oi8 = epp.tile([PART, 2 * C_OUT], i8, tag="oi8")
                nc.vector.tensor_scalar(out=oi8[:], in0=o[:], scalar1=Y_SCALE,
                                        scalar2=None, op0=ALU.mult)
                nc.sync.dma_start(y_d[b * PART:(b + 1) * PART, :], oi8[:])

            edge_phase(tableC, C_ROW, erC_sb, 40, 1, out_c)

    nc.compile()
    return nc


# ----------------------------------------------------------------------------
# entry point
# ----------------------------------------------------------------------------
def _get_compiled(K):
    if K not in _COMPILED:
        _COMPILED[K] = _build(K)
    return _COMPILED[K]


def kernel(**inputs):
    src = np.asarray(inputs["src"]).astype(np.int64) % N_NODES
    dst = np.asarray(inputs["dst"]).astype(np.int64) % N_NODES

    sched = _schedule(src, dst)
    in_maps = _prep_inputs(inputs, sched)
    nc = _get_compiled(sched["K"])

    from concourse.bass_utils import run_bass_kernel_spmd
    res = run_bass_kernel_spmd(nc, in_maps, list(range(N_CORES)))

    y0 = np.zeros((N_NODES, C_OUT), np.float32)
    y1 = np.zeros((N_NODES, C_OUT), np.float32)
    ncore = sched["node_core"]
    sic = sched["slot_in_core"]
    for c in range(N_CORES):
        nodes = np.where(ncore == c)[0]
        yc = res.results[c]["y"].astype(np.float32) / Y_SCALE
        y0[nodes] = yc[sic[nodes], 0:C_OUT]
        y1[nodes] = yc[sic[nodes], C_OUT:2 * C_OUT]
    out = np.stack([y0, y1], axis=0)
    _STATE["last"] = (nc, in_maps, sched)
    return out


# revision 3
# speedup vs baseline: 1.1475x; 1.1475x over previous
"""Trainium2 Bass kernel for a 2-branch stacked-GAT network (8 NeuronCores).

v2 design (optimized for axon dispatch wall-time):
  - Nodes are partitioned across the 8 cores (LPT by in-degree), then sorted
    by in-degree within each core so each 128-node block has a near-uniform
    max degree K[b].  Edge layout: partition = dst lane, free dim = that
    node's incoming edges (k-slots padded to K[b] with a sentinel src row).
  - Each GAT layer = node GEMM (fp16) -> all-gather of the fp16 feature
    table -> per-block edge softmax done entirely with free-dim reduces on
    the Vector engine (exact per-node max; no selection matrices, no PE work,
    no er gathers).  Messages are weighted in-place and reduced along k.
  - Pad k-slots gather a sentinel table row (feat=0, el=-60000) so their
    softmax weight underflows to 0 and their message contribution is 0.
  - All tables/weights/activations are fp16 (fp32 PSUM + fp32 softmax
    arithmetic); host I/O is fp16 everywhere; indices ship as int16.
  - A persistent jax compilation cache makes repeat dispatches skip XLA
    compilation of the (per-call rebuilt) shard_map closure.
"""

import os
import tempfile

import numpy as np

import sys
sys.path.insert(0, "/opt/trn_rl_repo")

# jax persistent compilation cache: run_bass_kernel_spmd builds a fresh jit
# closure per call; the persistent cache makes every call after the first a
# cache hit instead of a full XLA compile.
import jax

jax.config.update(
    "jax_compilation_cache_dir",
    os.path.join(tempfile.gettempdir(), "jaxcache_gat"),
)
jax.config.update("jax_persistent_cache_min_compile_time_secs", 0.0)
jax.config.update("jax_persistent_cache_min_entry_size_bytes", -1)

# ----------------------------------------------------------------------------
# problem constants (hardcoded per the task contract)
# ----------------------------------------------------------------------------
N_NODES = 50000
N_EDGES = 800000
IN_DIM = 256
C_OUT = 40
N_CORES = 8
PART = 128
BLOCKS = 49                      # 49 * 128 = 6272 node slots per core
SLOTS = BLOCKS * PART            # 6272
S_TOT = N_CORES * SLOTS          # 50176; sentinel row index == S_TOT
EL_SENT = -60000.0               # sentinel el (fp16-safe; exp underflows to 0)
IDX_OFF = 32768                  # int16 shipping offset for src indices

A_ROW, B_ROW, C_ROW = 520, 130, 82
Y_SCALE = 38.0                   # int8 output quantization scale
A_W, B_W, C_W = 528, 138, 90

_COMPILED = {}
_STATE = {}


# ----------------------------------------------------------------------------
# host-side graph scheduling
# ----------------------------------------------------------------------------
def _balanced_assign(weights, n_bins, cap):
    """Greedy LPT: heaviest item to lightest non-full bin. Returns bin ids."""
    import heapq

    order = np.argsort(-weights, kind="stable")
    loads = np.zeros(n_bins, dtype=np.int64)
    counts = np.zeros(n_bins, dtype=np.int64)
    out = np.empty(len(weights), dtype=np.int32)
    heap = [(0, b) for b in range(n_bins)]
    heapq.heapify(heap)
    for i in order:
        spill = []
        while True:
            load, b = heapq.heappop(heap)
            if counts[b] < cap:
                break
            spill.append((load, b))
        out[i] = b
        counts[b] += 1
        loads[b] += weights[i]
        heapq.heappush(heap, (loads[b], b))
        for item in spill:
            heapq.heappush(heap, item)
    return out


def _schedule(src, dst):
    """Shard nodes across cores; sort by degree into blocks; build edge slots."""
    deg = np.bincount(dst, minlength=N_NODES).astype(np.int64)
    node_core = _balanced_assign(deg, N_CORES, SLOTS)

    # slot within core = rank by degree (desc, stable)
    slot_in_core = np.zeros(N_NODES, dtype=np.int64)
    deg_sorted = np.zeros((N_CORES, SLOTS), dtype=np.int64)
    for c in range(N_CORES):
        nodes = np.where(node_core == c)[0]
        order = np.argsort(-deg[nodes], kind="stable")
        slot_in_core[nodes[order]] = np.arange(len(nodes))
        deg_sorted[c, : len(nodes)] = deg[nodes[order]]

    # K[b] = max in-degree in block b across all cores (sorted desc => first)
    K = deg_sorted[:, ::PART].max(axis=0)
    K = np.maximum(K, 1).astype(np.int64)
    off = np.zeros(BLOCKS + 1, dtype=np.int64)
    off[1:] = np.cumsum(K)
    sumk = int(off[-1])

    global_slot = node_core.astype(np.int64) * SLOTS + slot_in_core

    # per-edge placement: lane = slot%128, block = slot//128, k = rank in dst
    e_core = node_core[dst]
    e_slot = slot_in_core[dst]
    e_blk = e_slot // PART
    e_lane = e_slot % PART
    eorder = np.argsort(dst, kind="stable")
    dst_s = dst[eorder]
    grp_start = np.searchsorted(dst_s, np.arange(N_NODES), side="left")
    k_rank = np.arange(N_EDGES, dtype=np.int64) - grp_start[dst_s]
    kk = np.empty(N_EDGES, dtype=np.int64)
    kk[eorder] = k_rank

    sidx16 = np.full((N_CORES, PART, sumk), S_TOT - IDX_OFF, dtype=np.int16)
    col = off[e_blk] + kk
    sidx16[e_core, e_lane, col] = (global_slot[src] - IDX_OFF).astype(np.int16)

    return dict(
        K=tuple(int(k) for k in K),
        off=off,
        sumk=sumk,
        node_core=node_core,
        slot_in_core=slot_in_core,
        sidx16=sidx16,
    )


def _aug_w(W, al, ar):
    H, D = al.shape
    Wl = np.stack([W[:, h * D:(h + 1) * D] @ al[h] for h in range(H)], axis=1)
    Wr = np.stack([W[:, h * D:(h + 1) * D] @ ar[h] for h in range(H)], axis=1)
    return Wl.astype(np.float32), Wr.astype(np.float32)


def _prep_inputs(inputs, sched):
    """Build all per-core (and shared) device input arrays."""
    x = np.asarray(inputs["x"], np.float32)
    p32 = {k: np.asarray(v, np.float32) for k, v in inputs.items()
           if k not in ("x", "src", "dst")}

    # x in slot order, transposed for the GEMM lhsT, fp16
    xs = np.zeros((S_TOT, IN_DIM), np.float32)
    gs = sched["node_core"].astype(np.int64) * SLOTS + sched["slot_in_core"]
    xs[gs] = x
    xT = np.ascontiguousarray(
        xs.T.reshape(2, PART, S_TOT).transpose(1, 0, 2)).astype(np.float16)

    # layer-A weights [W00 256 | Wl0 4 | W10 256 | Wl1 4 | Wr0 4 | Wr1 4]
    Wl0, Wr0 = _aug_w(p32["W00"], p32["a00l"], p32["a00r"])
    Wl1, Wr1 = _aug_w(p32["W10"], p32["a10l"], p32["a10r"])
    WA = np.zeros((IN_DIM, A_W), np.float32)
    WA[:, 0:256] = p32["W00"]
    WA[:, 256:260] = Wl0
    WA[:, 260:516] = p32["W10"]
    WA[:, 516:520] = Wl1
    WA[:, 520:524] = Wr0
    WA[:, 524:528] = Wr1
    WA16 = np.ascontiguousarray(
        WA.reshape(2, PART, A_W).transpose(1, 0, 2)).astype(np.float16)

    # layer-B weights [W01 64 | Wl2 1 | W1f 64 | Wl5 1 | Wr2 1 | Wr5 1 | pad]
    Wl2, Wr2 = _aug_w(p32["W01"], p32["a01l"], p32["a01r"])
    Wl5, Wr5 = _aug_w(p32["W1f"], p32["a1fl"], p32["a1fr"])
    WB = np.zeros((512, B_W), np.float32)
    WB[0:256, 0:64] = p32["W01"]
    WB[0:256, 64:65] = Wl2
    WB[256:512, 65:129] = p32["W1f"]
    WB[256:512, 129:130] = Wl5
    WB[0:256, 130:131] = Wr2
    WB[256:512, 131:132] = Wr5
    WB16 = np.ascontiguousarray(
        WB.reshape(4, PART, B_W).transpose(1, 0, 2)).astype(np.float16)

    # layer-C weights [W0f 40 | Wl3 1 | W1o 40 | Wl6 1 | Wr3 1 | Wr6 1 | pad]
    Wl3, Wr3 = _aug_w(p32["W0f"], p32["a0fl"], p32["a0fr"])
    Wl6, Wr6 = _aug_w(p32["W1o"], p32["a1ol"], p32["a1or"])
    WC = np.zeros((PART, C_W), np.float32)
    WC[0:64, 0:40] = p32["W0f"]
    WC[0:64, 40:41] = Wl3
    WC[64:128, 41:81] = p32["W1o"]
    WC[64:128, 81:82] = Wl6
    WC[0:64, 82:83] = Wr3
    WC[64:128, 83:84] = Wr6
    WC16 = np.ascontiguousarray(
        WC.reshape(1, PART, C_W).transpose(1, 0, 2)).astype(np.float16)

    # pack everything into ONE fp16 blob per core: fewer axon transfers
    # (per-transfer overhead dominates the upload path).  The replicated
    # weight stack [128, 1698] is sharded: each core ships 16 partition-rows
    # and the device all-gathers them back.
    wstack = np.zeros((PART, 1704), np.float16)   # 1698 cols padded to /8
    wstack[:, 0:2 * A_W] = WA16.reshape(PART, -1)
    wstack[:, 2 * A_W:2 * A_W + 4 * B_W] = WB16.reshape(PART, -1)
    wstack[:, 2 * A_W + 4 * B_W:2 * A_W + 4 * B_W + C_W] = WC16.reshape(PART, -1)
    rpc = PART // N_CORES
    in_maps = []
    for c in range(N_CORES):
        wpart = wstack[c * rpc:(c + 1) * rpc].reshape(PART, -1)  # [128, 213]
        blob = np.concatenate([
            xT[:, :, c * SLOTS:(c + 1) * SLOTS].reshape(PART, -1),
            wpart,
            sched["sidx16"][c].view(np.float16),
        ], axis=1)
        in_maps.append(dict(blob=np.ascontiguousarray(blob)))
    return in_maps


# ----------------------------------------------------------------------------
# device program
# ----------------------------------------------------------------------------
def _build(K):
    import concourse.bass as bass
    import concourse.tile as tile
    from concourse import bacc, mybir

    f32 = mybir.dt.float32
    f16 = mybir.dt.float16
    i32 = mybir.dt.int32
    i16 = mybir.dt.int16
    i8 = mybir.dt.int8
    ALU = mybir.AluOpType
    ACT = mybir.ActivationFunctionType
    X = mybir.AxisListType.X

    off = np.zeros(BLOCKS + 1, dtype=np.int64)
    off[1:] = np.cumsum(np.asarray(K))
    sumk = int(off[-1])
    kmax = int(max(K))

    nc = bacc.Bacc("TRN2", target_bir_lowering=False, debug=False,
                   num_devices=N_CORES)

    # ---- I/O (single packed fp16 blob per core) ----
    W_WP = 1704                          # weight cols (1698 padded to /8)
    RPC = PART // N_CORES                # weight rows shipped per core
    WCOLS = RPC * W_WP // PART           # 213 blob cols for the weight shard
    OFF_X = 0
    OFF_W = OFF_X + 2 * SLOTS
    OFF_SIDX = OFF_W + WCOLS
    BLOB_W = OFF_SIDX + sumk
    blob = nc.dram_tensor("blob", [PART, BLOB_W], f16, kind="ExternalInput")
    xTl16 = blob[:, OFF_X:OFF_W].rearrange("p (k s) -> p k s", k=2)
    y_d = nc.dram_tensor("y", [SLOTS, 2 * C_OUT], i8, kind="ExternalOutput")

    # ---- internal DRAM ----
    tbA_sh = nc.dram_tensor("tbA_sh", [SLOTS, A_ROW], f16)
    tableA = nc.dram_tensor("tableA", [S_TOT + 1, A_ROW], f16,
                            addr_space="Shared")
    aggA = nc.dram_tensor("aggA", [SLOTS, 512], f16)
    tbB_sh = nc.dram_tensor("tbB_sh", [SLOTS, B_ROW], f16)
    tableB = nc.dram_tensor("tableB", [S_TOT + 1, B_ROW], f16,
                            addr_space="Shared")
    aggB = nc.dram_tensor("aggB", [SLOTS, PART], f16)
    tbC_sh = nc.dram_tensor("tbC_sh", [SLOTS, C_ROW], f16)
    tableC = nc.dram_tensor("tableC", [S_TOT + 1, C_ROW], f16,
                            addr_space="Shared")
    w_sh = nc.dram_tensor("w_sh", [RPC, W_WP], f16)
    w_full = nc.dram_tensor("w_full", [PART, W_WP], f16, addr_space="Shared")

    groups = [list(range(N_CORES))]

    with tile.TileContext(nc, trace_sim=False) as tc:
        with tc.tile_pool(name="const", bufs=1) as cpool, \
             tc.tile_pool(name="gemm_in", bufs=3) as gip, \
             tc.tile_pool(name="gemm_out", bufs=3) as gop, \
             tc.tile_pool(name="gath", bufs=2) as gap, \
             tc.tile_pool(name="small", bufs=3) as smp, \
             tc.tile_pool(name="epi", bufs=2) as epp:

            # ---- constants / resident tiles ----
            # weights: each core shipped RPC partition-rows (flattened to
            # [128, WCOLS] in the blob); stage to DRAM, all-gather the full
            # [128, W_WP] stack, then load to SBUF
            wp = cpool.tile([PART, WCOLS], f16)
            nc.sync.dma_start(wp[:], blob[:, OFF_W:OFF_SIDX])
            nc.sync.dma_start(
                w_sh[:, :].rearrange("r w -> (r w)").rearrange(
                    "(p q) -> p q", p=PART), wp[:])
            nc.gpsimd.collective_compute(
                "AllGather", ALU.bypass, replica_groups=groups,
                ins=[w_sh[:, :]], outs=[w_full[:, :]])
            wa_t = cpool.tile([PART, 2, A_W], f16)
            nc.sync.dma_start(wa_t[:].rearrange("p k w -> p (k w)"),
                              w_full[:, 0:2 * A_W])
            wb_t = cpool.tile([PART, 4, B_W], f16)
            nc.sync.dma_start(wb_t[:].rearrange("p k w -> p (k w)"),
                              w_full[:, 2 * A_W:2 * A_W + 4 * B_W])
            wc_t = cpool.tile([PART, 1, C_W], f16)
            nc.sync.dma_start(wc_t[:].rearrange("p k w -> p (k w)"),
                              w_full[:, 2 * A_W + 4 * B_W:
                                     2 * A_W + 4 * B_W + C_W])

            s16 = cpool.tile([PART, sumk], i16)
            nc.sync.dma_start(s16[:], blob[:, OFF_SIDX:BLOB_W].bitcast(i16))
            s32 = cpool.tile([PART, sumk], i32)
            nc.vector.tensor_copy(s32[:], s16[:])
            nc.vector.tensor_scalar(out=s32[:], in0=s32[:], scalar1=IDX_OFF,
                                    scalar2=None, op0=ALU.add)

            erA_sb = cpool.tile([PART, BLOCKS, 8], f16)
            erB_sb = cpool.tile([PART, BLOCKS, 2], f16)
            erC_sb = cpool.tile([PART, BLOCKS, 2], f16)

            # sentinel rows (feat 0, el slots EL_SENT)
            sentA = cpool.tile([1, A_ROW], f16)
            nc.vector.memset(sentA[:], 0.0)
            nc.vector.memset(sentA[:, 256:260], EL_SENT)
            nc.vector.memset(sentA[:, 516:520], EL_SENT)
            nc.sync.dma_start(tableA[S_TOT:S_TOT + 1, :], sentA[:])
            sentB = cpool.tile([1, B_ROW], f16)
            nc.vector.memset(sentB[:], 0.0)
            nc.vector.memset(sentB[:, 64:65], EL_SENT)
            nc.vector.memset(sentB[:, 129:130], EL_SENT)
            nc.sync.dma_start(tableB[S_TOT:S_TOT + 1, :], sentB[:])
            sentC = cpool.tile([1, C_ROW], f16)
            nc.vector.memset(sentC[:], 0.0)
            nc.vector.memset(sentC[:, 40:41], EL_SENT)
            nc.vector.memset(sentC[:, 81:82], EL_SENT)
            nc.sync.dma_start(tableC[S_TOT:S_TOT + 1, :], sentC[:])

            # ---- phase 1: GEMM-A (node-sharded) + all-gather ----
            with tc.tile_pool(name="psA", bufs=2, space="PSUM") as gpp:
                for b in range(BLOCKS):
                    sl = slice(b * PART, (b + 1) * PART)
                    xt = gip.tile([PART, 2, PART], f16, tag="xt")
                    nc.sync.dma_start(xt[:], xTl16[:, :, sl])
                    ps = gpp.tile([PART, A_W], f32, space="PSUM", tag="psA")
                    for k2 in range(2):
                        nc.tensor.matmul(ps[:, 0:512], lhsT=xt[:, k2, :],
                                         rhs=wa_t[:, k2, 0:512],
                                         start=(k2 == 0), stop=(k2 == 1))
                        nc.tensor.matmul(ps[:, 512:A_W], lhsT=xt[:, k2, :],
                                         rhs=wa_t[:, k2, 512:A_W],
                                         start=(k2 == 0), stop=(k2 == 1))
                    row = gop.tile([PART, A_ROW], f16, tag="rowA")
                    if b % 2 == 0:
                        nc.vector.tensor_copy(row[:], ps[:, 0:A_ROW])
                    else:
                        nc.scalar.copy(row[:], ps[:, 0:A_ROW])
                    nc.vector.tensor_copy(erA_sb[:, b, :], ps[:, A_ROW:A_W])
                    nc.sync.dma_start(tbA_sh[sl, :], row[:])
            nc.gpsimd.collective_compute(
                "AllGather", ALU.bypass, replica_groups=groups,
                ins=[tbA_sh[:, :]], outs=[tableA[0:S_TOT, :]])

            # ---- edge phase helper ----
            def edge_phase(table, row_w, er_sb, fdim, nheads, out_cb):
                """One GAT aggregation layer over all blocks (both branches).

                row layout per branch: [feat fdim*nheads | el nheads]
                """
                fw = fdim * nheads
                r = fw + nheads
                for b in range(BLOCKS):
                    kb = K[b]
                    g = gap.tile([PART, kb, row_w], f16, tag="g")
                    for k in range(kb):
                        c0 = int(off[b]) + k
                        nc.gpsimd.indirect_dma_start(
                            out=g[:, k, :], out_offset=None,
                            in_=table[:, :],
                            in_offset=bass.IndirectOffsetOnAxis(
                                ap=s32[:, c0:c0 + 1], axis=0))
                    # e = el + er  [128, kb, 2, H] f32
                    el = g[:].rearrange("p k (b2 r) -> p k b2 r", b2=2)[
                        :, :, :, fw:fw + nheads]
                    e = smp.tile([PART, kb, 2, nheads], f32, tag="e")
                    erv = er_sb[:, b, :].rearrange(
                        "p (o b2 h) -> p o b2 h", o=1, b2=2)
                    nc.vector.tensor_tensor(
                        out=e[:], in0=el,
                        in1=erv.to_broadcast([PART, kb, 2, nheads]),
                        op=ALU.add)
                    # m = lrelu(max_k e); e = lrelu(e) - m
                    # (ACT.Lrelu ignores alpha and uses slope 0.01, so leaky
                    # relu is computed manually: max(x, 0.2*x))
                    m = smp.tile([PART, 1, 2, nheads], f32, tag="m")
                    nc.vector.tensor_reduce(
                        out=m[:], in_=e[:].rearrange("p k b2 h -> p b2 h k"),
                        axis=X, op=ALU.max)
                    m2 = smp.tile([PART, 1, 2, nheads], f32, tag="m2")
                    nc.vector.tensor_scalar(out=m2[:], in0=m[:], scalar1=0.2,
                                            scalar2=None, op0=ALU.mult)
                    nc.vector.tensor_tensor(out=m[:], in0=m[:], in1=m2[:],
                                            op=ALU.max)
                    e2 = smp.tile([PART, kb, 2, nheads], f32, tag="e2")
                    nc.vector.tensor_scalar(out=e2[:], in0=e[:], scalar1=0.2,
                                            scalar2=None, op0=ALU.mult)
                    nc.vector.tensor_tensor(out=e[:], in0=e[:], in1=e2[:],
                                            op=ALU.max)
                    nc.vector.tensor_tensor(
                        out=e[:], in0=e[:],
                        in1=m[:].to_broadcast([PART, kb, 2, nheads]),
                        op=ALU.subtract)
                    ex = smp.tile([PART, kb, 2, nheads], f16, tag="ex")
                    nc.scalar.activation(ex[:], e[:], ACT.Exp)
                    # den / reciprocal
                    den = smp.tile([PART, 1, 2, nheads], f32, tag="den")
                    nc.vector.tensor_reduce(
                        out=den[:], in_=ex[:].rearrange("p k b2 h -> p b2 h k"),
                        axis=X, op=ALU.add)
                    nc.vector.tensor_scalar(out=den[:], in0=den[:],
                                            scalar1=1e-9, scalar2=None,
                                            op0=ALU.max)
                    rec = smp.tile([PART, 1, 2, nheads], f32, tag="rec")
                    nc.vector.reciprocal(rec[:], den[:])
                    # g(feat) *= ex
                    gf = g[:].rearrange("p k (b2 r) -> p k b2 r", b2=2)[
                        :, :, :, 0:fw].rearrange(
                        "p k b2 (h d) -> p k b2 h d", h=nheads)
                    exb = ex[:].rearrange(
                        "p k b2 (h o) -> p k b2 h o", o=1).to_broadcast(
                        [PART, kb, 2, nheads, fdim])
                    nc.vector.tensor_tensor(out=gf, in0=gf, in1=exb,
                                            op=ALU.mult)
                    # msum over k -> [128, 2, H, fdim] f32; rst = msum * rec
                    ms = epp.tile([PART, 2, nheads, fdim], f32, tag="ms")
                    gfk = g[:].rearrange("p k (b2 r) -> p k b2 r", b2=2)[
                        :, :, :, 0:fw].rearrange(
                        "p k b2 (h d) -> p b2 h d k", h=nheads)
                    nc.vector.tensor_reduce(out=ms[:], in_=gfk, axis=X,
                                            op=ALU.add)
                    o = epp.tile([PART, 2 * fw], f32, tag="o")
                    ov = o[:].rearrange("p (b2 h d) -> p b2 h d", b2=2,
                                        h=nheads)
                    recb = rec[:].rearrange(
                        "p o b2 (h o2) -> p o b2 h o2", o2=1)[:, 0]
                    nc.vector.tensor_tensor(
                        out=ov, in0=ms[:],
                        in1=recb.to_broadcast([PART, 2, nheads, fdim]),
                        op=ALU.mult)
                    out_cb(b, o)

            def elu_inplace(ap, width):
                """ap <- elu(ap): relu(x) + exp(min(x,0)) - 1."""
                tm = epp.tile([PART, width], f32, tag="elu_t")
                nc.vector.tensor_scalar(out=tm[:], in0=ap, scalar1=0.0,
                                        scalar2=None, op0=ALU.min)
                te = epp.tile([PART, width], f32, tag="elu_e")
                nc.scalar.activation(te[:], tm[:], ACT.Exp)
                nc.scalar.activation(tm[:], ap, ACT.Relu)
                nc.vector.tensor_tensor(out=te[:], in0=te[:], in1=tm[:],
                                        op=ALU.add)
                nc.vector.tensor_scalar(out=ap, in0=te[:], scalar1=-1.0,
                                        scalar2=None, op0=ALU.add)

            # ---- edge phase A -> aggA ----
            def out_a(b, o):
                elu_inplace(o[:], 512)
                o16 = epp.tile([PART, 512], f16, tag="o16")
                nc.scalar.copy(o16[:], o[:])
                nc.sync.dma_start(aggA[b * PART:(b + 1) * PART, :], o16[:])

            edge_phase(tableA, A_ROW, erA_sb, 64, 4, out_a)

            # ---- phase 3: GEMM-B (sharded) + all-gather ----
            with tc.tile_pool(name="psB", bufs=2, space="PSUM") as gpp:
                for b in range(BLOCKS):
                    sl = slice(b * PART, (b + 1) * PART)
                    hT = gip.tile([PART, 4, PART], f16, tag="hT")
                    for k2 in range(4):
                        nc.sync.dma_start_transpose(
                            hT[:, k2, :],
                            aggA[sl, k2 * PART:(k2 + 1) * PART])
                    psb = gpp.tile([PART, B_W], f32, space="PSUM", tag="psB")
                    for k2 in range(4):
                        nc.tensor.matmul(psb[:], lhsT=hT[:, k2, :],
                                         rhs=wb_t[:, k2, :],
                                         start=(k2 == 0), stop=(k2 == 3))
                    rowb = gop.tile([PART, B_ROW], f16, tag="rowB")
                    nc.vector.tensor_copy(rowb[:], psb[:, 0:B_ROW])
                    nc.vector.tensor_copy(erB_sb[:, b, :], psb[:, 130:132])
                    nc.sync.dma_start(tbB_sh[sl, :], rowb[:])
            nc.gpsimd.collective_compute(
                "AllGather", ALU.bypass, replica_groups=groups,
                ins=[tbB_sh[:, :]], outs=[tableB[0:S_TOT, :]])

            # ---- edge phase B -> aggB ----
            def out_b(b, o):
                elu_inplace(o[:, 0:64], 64)
                o16 = epp.tile([PART, PART], f16, tag="o16b")
                nc.scalar.copy(o16[:], o[:])
                nc.sync.dma_start(aggB[b * PART:(b + 1) * PART, :], o16[:])

            edge_phase(tableB, B_ROW, erB_sb, 64, 1, out_b)

            # ---- phase 5: GEMM-C (sharded) + all-gather ----
            with tc.tile_pool(name="psC", bufs=2, space="PSUM") as gpp:
                for b in range(BLOCKS):
                    sl = slice(b * PART, (b + 1) * PART)
                    hc = gip.tile([PART, PART], f16, tag="hc")
                    nc.sync.dma_start_transpose(hc[:], aggB[sl, :])
                    psc = gpp.tile([PART, C_W], f32, space="PSUM", tag="psC")
                    nc.tensor.matmul(psc[:], lhsT=hc[:], rhs=wc_t[:, 0, :],
                                     start=True, stop=True)
                    rowc = gop.tile([PART, C_ROW], f16, tag="rowC")
                    nc.vector.tensor_copy(rowc[:], psc[:, 0:C_ROW])
                    nc.vector.tensor_copy(erC_sb[:, b, :], psc[:, 82:84])
                    nc.sync.dma_start(tbC_sh[sl, :], rowc[:])
            nc.gpsimd.collective_compute(
                "AllGather", ALU.bypass, replica_groups=groups,
                ins=[tbC_sh[:, :]], outs=[tableC[0:S_TOT, :]])

            # ---- edge phase C -> y ----
            def out_c(b, o):
                elu_inplace(o[:, 40:80], 40)
                # int8 output: y in [-3.37, 3.37], quant step 1/38 (rounds+
                # saturates on conversion); host divides by Y_SCALE
                oi8 = epp.tile([PART, 2 * C_OUT], i8, tag="oi8")
                nc.vector.tensor_scalar(out=oi8[:], in0=o[:], scalar1=Y_SCALE,
                                        scalar2=None, op0=ALU.mult)
                nc.sync.dma_start(y_d[b * PART:(b + 1) * PART, :], oi8[:])

            edge_phase(tableC, C_ROW, erC_sb, 40, 1, out_c)

    nc.compile()
    return nc


# ----------------------------------------------------------------------------
# entry point
# ----------------------------------------------------------------------------
def _get_compiled(K):
    if K not in _COMPILED:
        _COMPILED[K] = _build(K)
    return _COMPILED[K]


def kernel(**inputs):
    src = np.asarray(inputs["src"]).astype(np.int64) % N_NODES
    dst = np.asarray(inputs["dst"]).astype(np.int64) % N_NODES

    sched = _schedule(src, dst)
    in_maps = _prep_inputs(inputs, sched)
    nc = _get_compiled(sched["K"])

    from concourse.bass_utils import run_bass_kernel_spmd
    res = run_bass_kernel_spmd(nc, in_maps, list(range(N_CORES)))

    y0 = np.zeros((N_NODES, C_OUT), np.float32)
    y1 = np.zeros((N_NODES, C_OUT), np.float32)
    ncore = sched["node_core"]
    sic = sched["slot_in_core"]
    for c in range(N_CORES):
        nodes = np.where(ncore == c)[0]
        yc = res.results[c]["y"].astype(np.float32) / Y_SCALE
        y0[nodes] = yc[sic[nodes], 0:C_OUT]
        y1[nodes] = yc[sic[nodes], C_OUT:2 * C_OUT]
    out = np.stack([y0, y1], axis=0)
    _STATE["last"] = (nc, in_maps, sched)
    return out


# revision 4
# speedup vs baseline: 1.2044x; 1.0496x over previous
"""Trainium2 Bass kernel for a 2-branch stacked-GAT network (8 NeuronCores).

v2 design (optimized for axon dispatch wall-time):
  - Nodes are partitioned across the 8 cores (LPT by in-degree), then sorted
    by in-degree within each core so each 128-node block has a near-uniform
    max degree K[b].  Edge layout: partition = dst lane, free dim = that
    node's incoming edges (k-slots padded to K[b] with a sentinel src row).
  - Each GAT layer = node GEMM (fp16) -> all-gather of the fp16 feature
    table -> per-block edge softmax done entirely with free-dim reduces on
    the Vector engine (exact per-node max; no selection matrices, no PE work,
    no er gathers).  Messages are weighted in-place and reduced along k.
  - Pad k-slots gather a sentinel table row (feat=0, el=-60000) so their
    softmax weight underflows to 0 and their message contribution is 0.
  - All tables/weights/activations are fp16 (fp32 PSUM + fp32 softmax
    arithmetic); host I/O is fp16 everywhere; indices ship as int16.
  - A persistent jax compilation cache makes repeat dispatches skip XLA
    compilation of the (per-call rebuilt) shard_map closure.
"""

import os
import tempfile

import numpy as np

import sys
sys.path.insert(0, "/opt/trn_rl_repo")

# jax persistent compilation cache: run_bass_kernel_spmd builds a fresh jit
# closure per call; the persistent cache makes every call after the first a
# cache hit instead of a full XLA compile.
import jax

jax.config.update(
    "jax_compilation_cache_dir",
    os.path.join(tempfile.gettempdir(), "jaxcache_gat"),
)
jax.config.update("jax_persistent_cache_min_compile_time_secs", 0.0)
jax.config.update("jax_persistent_cache_min_entry_size_bytes", -1)

# ----------------------------------------------------------------------------
# problem constants (hardcoded per the task contract)
# ----------------------------------------------------------------------------
N_NODES = 50000
N_EDGES = 800000
IN_DIM = 256
C_OUT = 40
N_CORES = 8
PART = 128
BLOCKS = 49                      # 49 * 128 = 6272 node slots per core
SLOTS = BLOCKS * PART            # 6272
S_TOT = N_CORES * SLOTS          # 50176; sentinel row index == S_TOT
EL_SENT = -60000.0               # sentinel el (fp16-safe; exp underflows to 0)
IDX_OFF = 32768                  # int16 shipping offset for src indices

A_ROW, B_ROW, C_ROW = 520, 130, 82
Y_SCALE = 38.0                   # int8 output quantization scale
A_W, B_W, C_W = 528, 138, 90

_COMPILED = {}
_STATE = {}


# ----------------------------------------------------------------------------
# host-side graph scheduling
# ----------------------------------------------------------------------------
def _balanced_assign(weights, n_bins, cap):
    """Greedy LPT: heaviest item to lightest non-full bin. Returns bin ids."""
    import heapq

    order = np.argsort(-weights, kind="stable")
    loads = np.zeros(n_bins, dtype=np.int64)
    counts = np.zeros(n_bins, dtype=np.int64)
    out = np.empty(len(weights), dtype=np.int32)
    heap = [(0, b) for b in range(n_bins)]
    heapq.heapify(heap)
    for i in order:
        spill = []
        while True:
            load, b = heapq.heappop(heap)
            if counts[b] < cap:
                break
            spill.append((load, b))
        out[i] = b
        counts[b] += 1
        loads[b] += weights[i]
        heapq.heappush(heap, (loads[b], b))
        for item in spill:
            heapq.heappush(heap, item)
    return out


def _schedule(src, dst):
    """Shard nodes across cores; sort by degree into blocks; build edge slots."""
    deg = np.bincount(dst, minlength=N_NODES).astype(np.int64)
    node_core = _balanced_assign(deg, N_CORES, SLOTS)

    # slot within core = rank by degree (desc, stable)
    slot_in_core = np.zeros(N_NODES, dtype=np.int64)
    deg_sorted = np.zeros((N_CORES, SLOTS), dtype=np.int64)
    for c in range(N_CORES):
        nodes = np.where(node_core == c)[0]
        order = np.argsort(-deg[nodes], kind="stable")
        slot_in_core[nodes[order]] = np.arange(len(nodes))
        deg_sorted[c, : len(nodes)] = deg[nodes[order]]

    # K[b] = max in-degree in block b across all cores (sorted desc => first)
    K = deg_sorted[:, ::PART].max(axis=0)
    K = np.maximum(K, 1).astype(np.int64)
    off = np.zeros(BLOCKS + 1, dtype=np.int64)
    off[1:] = np.cumsum(K)
    sumk = int(off[-1])

    global_slot = node_core.astype(np.int64) * SLOTS + slot_in_core

    # per-edge placement: lane = slot%128, block = slot//128, k = rank in dst
    e_core = node_core[dst]
    e_slot = slot_in_core[dst]
    e_blk = e_slot // PART
    e_lane = e_slot % PART
    eorder = np.argsort(dst, kind="stable")
    dst_s = dst[eorder]
    grp_start = np.searchsorted(dst_s, np.arange(N_NODES), side="left")
    k_rank = np.arange(N_EDGES, dtype=np.int64) - grp_start[dst_s]
    kk = np.empty(N_EDGES, dtype=np.int64)
    kk[eorder] = k_rank

    sidx16 = np.full((N_CORES, PART, sumk), S_TOT - IDX_OFF, dtype=np.int16)
    col = off[e_blk] + kk
    sidx16[e_core, e_lane, col] = (global_slot[src] - IDX_OFF).astype(np.int16)

    return dict(
        K=tuple(int(k) for k in K),
        off=off,
        sumk=sumk,
        node_core=node_core,
        slot_in_core=slot_in_core,
        sidx16=sidx16,
    )


def _aug_w(W, al, ar):
    H, D = al.shape
    Wl = np.stack([W[:, h * D:(h + 1) * D] @ al[h] for h in range(H)], axis=1)
    Wr = np.stack([W[:, h * D:(h + 1) * D] @ ar[h] for h in range(H)], axis=1)
    return Wl.astype(np.float32), Wr.astype(np.float32)


def _prep_inputs(inputs, sched):
    """Build all per-core (and shared) device input arrays."""
    x = np.asarray(inputs["x"], np.float32)
    p32 = {k: np.asarray(v, np.float32) for k, v in inputs.items()
           if k not in ("x", "src", "dst")}

    # x in slot order, transposed for the GEMM lhsT, 12-bit packed:
    # per-node scale s = absmax/2047, xq = round(x/s) + 2048 in [1, 4095];
    # node pairs packed into 3 bytes. The scale is folded back into the
    # PSUM->row copy on device (it factors out of the GEMM row exactly).
    s_row = np.maximum(np.abs(x).max(axis=1), 1e-6) / 2047.0
    xq = (np.clip(np.round(x / s_row[:, None]), -2047, 2047)
          .astype(np.int32) + 2048)
    xs = np.zeros((S_TOT, IN_DIM), np.int32)
    xs[:] = 2048
    gs = sched["node_core"].astype(np.int64) * SLOTS + sched["slot_in_core"]
    xs[gs] = xq
    xT = np.ascontiguousarray(
        xs.T.reshape(2, PART, S_TOT).transpose(1, 0, 2))       # [128,2,S_TOT]
    v0, v1 = xT[:, :, 0::2], xT[:, :, 1::2]
    pk = np.zeros((PART, 2, 3 * (S_TOT // 2)), np.uint8)
    pk[:, :, 0::3] = v0 & 255
    pk[:, :, 1::3] = (v0 >> 8) | ((v1 & 15) << 4)
    pk[:, :, 2::3] = v1 >> 4
    s_slot = np.zeros(S_TOT, np.float16)
    s_slot[gs] = s_row.astype(np.float16)
    # per-core scale tile layout [lane 128, block 49]
    s_tiles = s_slot.reshape(N_CORES, BLOCKS, PART).transpose(0, 2, 1)

    # layer-A weights [W00 256 | Wl0 4 | W10 256 | Wl1 4 | Wr0 4 | Wr1 4]
    Wl0, Wr0 = _aug_w(p32["W00"], p32["a00l"], p32["a00r"])
    Wl1, Wr1 = _aug_w(p32["W10"], p32["a10l"], p32["a10r"])
    WA = np.zeros((IN_DIM, A_W), np.float32)
    WA[:, 0:256] = p32["W00"]
    WA[:, 256:260] = Wl0
    WA[:, 260:516] = p32["W10"]
    WA[:, 516:520] = Wl1
    WA[:, 520:524] = Wr0
    WA[:, 524:528] = Wr1
    WA16 = np.ascontiguousarray(
        WA.reshape(2, PART, A_W).transpose(1, 0, 2)).astype(np.float16)

    # layer-B weights [W01 64 | Wl2 1 | W1f 64 | Wl5 1 | Wr2 1 | Wr5 1 | pad]
    Wl2, Wr2 = _aug_w(p32["W01"], p32["a01l"], p32["a01r"])
    Wl5, Wr5 = _aug_w(p32["W1f"], p32["a1fl"], p32["a1fr"])
    WB = np.zeros((512, B_W), np.float32)
    WB[0:256, 0:64] = p32["W01"]
    WB[0:256, 64:65] = Wl2
    WB[256:512, 65:129] = p32["W1f"]
    WB[256:512, 129:130] = Wl5
    WB[0:256, 130:131] = Wr2
    WB[256:512, 131:132] = Wr5
    WB16 = np.ascontiguousarray(
        WB.reshape(4, PART, B_W).transpose(1, 0, 2)).astype(np.float16)

    # layer-C weights [W0f 40 | Wl3 1 | W1o 40 | Wl6 1 | Wr3 1 | Wr6 1 | pad]
    Wl3, Wr3 = _aug_w(p32["W0f"], p32["a0fl"], p32["a0fr"])
    Wl6, Wr6 = _aug_w(p32["W1o"], p32["a1ol"], p32["a1or"])
    WC = np.zeros((PART, C_W), np.float32)
    WC[0:64, 0:40] = p32["W0f"]
    WC[0:64, 40:41] = Wl3
    WC[64:128, 41:81] = p32["W1o"]
    WC[64:128, 81:82] = Wl6
    WC[0:64, 82:83] = Wr3
    WC[64:128, 83:84] = Wr6
    WC16 = np.ascontiguousarray(
        WC.reshape(1, PART, C_W).transpose(1, 0, 2)).astype(np.float16)

    # pack everything into ONE fp16 blob per core: fewer axon transfers
    # (per-transfer overhead dominates the upload path).  The replicated
    # weight stack [128, 1698] is sharded: each core ships 16 partition-rows
    # and the device all-gathers them back.
    wstack = np.zeros((PART, 1704), np.float16)   # 1698 cols padded to /8
    wstack[:, 0:2 * A_W] = WA16.reshape(PART, -1)
    wstack[:, 2 * A_W:2 * A_W + 4 * B_W] = WB16.reshape(PART, -1)
    wstack[:, 2 * A_W + 4 * B_W:2 * A_W + 4 * B_W + C_W] = WC16.reshape(PART, -1)
    rpc = PART // N_CORES
    xcols = 3 * (SLOTS // 2)                     # packed bytes per k-half
    in_maps = []
    for c in range(N_CORES):
        wpart = wstack[c * rpc:(c + 1) * rpc].reshape(PART, -1)  # [128, 213]
        blob = np.concatenate([
            pk[:, :, c * xcols:(c + 1) * xcols].reshape(PART, -1)
              .view(np.float16),
            np.ascontiguousarray(s_tiles[c]),
            wpart,
            sched["sidx16"][c].view(np.float16),
        ], axis=1)
        in_maps.append(dict(blob=np.ascontiguousarray(blob)))
    return in_maps


# ----------------------------------------------------------------------------
# device program
# ----------------------------------------------------------------------------
def _build(K):
    import concourse.bass as bass
    import concourse.tile as tile
    from concourse import bacc, mybir

    f32 = mybir.dt.float32
    f16 = mybir.dt.float16
    i32 = mybir.dt.int32
    i16 = mybir.dt.int16
    i8 = mybir.dt.int8
    u8 = mybir.dt.uint8
    ALU = mybir.AluOpType
    ACT = mybir.ActivationFunctionType
    X = mybir.AxisListType.X

    off = np.zeros(BLOCKS + 1, dtype=np.int64)
    off[1:] = np.cumsum(np.asarray(K))
    sumk = int(off[-1])
    kmax = int(max(K))

    nc = bacc.Bacc("TRN2", target_bir_lowering=False, debug=False,
                   num_devices=N_CORES)

    # ---- I/O (single packed fp16 blob per core) ----
    W_WP = 1704                          # weight cols (1698 padded to /8)
    RPC = PART // N_CORES                # weight rows shipped per core
    WCOLS = RPC * W_WP // PART           # 213 blob cols for the weight shard
    XCOLS = 3 * (SLOTS // 2) // 2        # packed-x f16 cols per k-half (4704)
    OFF_X = 0
    OFF_S = OFF_X + 2 * XCOLS
    OFF_W = OFF_S + BLOCKS
    OFF_SIDX = OFF_W + WCOLS
    BLOB_W = OFF_SIDX + sumk
    blob = nc.dram_tensor("blob", [PART, BLOB_W], f16, kind="ExternalInput")
    xpk = blob[:, OFF_X:OFF_S].bitcast(u8).rearrange(
        "p (k b) -> p k b", k=2)         # [128, 2, 3*SLOTS/2] bytes
    y_d = nc.dram_tensor("y", [SLOTS, 2 * C_OUT], i8, kind="ExternalOutput")

    # ---- internal DRAM ----
    tbA_sh = nc.dram_tensor("tbA_sh", [SLOTS, A_ROW], f16)
    tableA = nc.dram_tensor("tableA", [S_TOT + 1, A_ROW], f16,
                            addr_space="Shared")
    aggA = nc.dram_tensor("aggA", [SLOTS, 512], f16)
    tbB_sh = nc.dram_tensor("tbB_sh", [SLOTS, B_ROW], f16)
    tableB = nc.dram_tensor("tableB", [S_TOT + 1, B_ROW], f16,
                            addr_space="Shared")
    aggB = nc.dram_tensor("aggB", [SLOTS, PART], f16)
    tbC_sh = nc.dram_tensor("tbC_sh", [SLOTS, C_ROW], f16)
    tableC = nc.dram_tensor("tableC", [S_TOT + 1, C_ROW], f16,
                            addr_space="Shared")
    w_sh = nc.dram_tensor("w_sh", [RPC, W_WP], f16)
    w_full = nc.dram_tensor("w_full", [PART, W_WP], f16, addr_space="Shared")

    groups = [list(range(N_CORES))]

    with tile.TileContext(nc, trace_sim=False) as tc:
        with tc.tile_pool(name="const", bufs=1) as cpool, \
             tc.tile_pool(name="gemm_in", bufs=3) as gip, \
             tc.tile_pool(name="gemm_out", bufs=3) as gop, \
             tc.tile_pool(name="gath", bufs=2) as gap, \
             tc.tile_pool(name="small", bufs=3) as smp, \
             tc.tile_pool(name="epi", bufs=2) as epp:

            # ---- constants / resident tiles ----
            # weights: each core shipped RPC partition-rows (flattened to
            # [128, WCOLS] in the blob); stage to DRAM, all-gather the full
            # [128, W_WP] stack, then load to SBUF
            wp = cpool.tile([PART, WCOLS], f16)
            nc.sync.dma_start(wp[:], blob[:, OFF_W:OFF_SIDX])
            nc.sync.dma_start(
                w_sh[:, :].rearrange("r w -> (r w)").rearrange(
                    "(p q) -> p q", p=PART), wp[:])
            nc.gpsimd.collective_compute(
                "AllGather", ALU.bypass, replica_groups=groups,
                ins=[w_sh[:, :]], outs=[w_full[:, :]])
            wa_t = cpool.tile([PART, 2, A_W], f16)
            nc.sync.dma_start(wa_t[:].rearrange("p k w -> p (k w)"),
                              w_full[:, 0:2 * A_W])
            wb_t = cpool.tile([PART, 4, B_W], f16)
            nc.sync.dma_start(wb_t[:].rearrange("p k w -> p (k w)"),
                              w_full[:, 2 * A_W:2 * A_W + 4 * B_W])
            wc_t = cpool.tile([PART, 1, C_W], f16)
            nc.sync.dma_start(wc_t[:].rearrange("p k w -> p (k w)"),
                              w_full[:, 2 * A_W + 4 * B_W:
                                     2 * A_W + 4 * B_W + C_W])

            s16 = cpool.tile([PART, sumk], i16)
            nc.sync.dma_start(s16[:], blob[:, OFF_SIDX:BLOB_W].bitcast(i16))
            sx_t = cpool.tile([PART, BLOCKS], f16)
            nc.sync.dma_start(sx_t[:], blob[:, OFF_S:OFF_W])
            s32 = cpool.tile([PART, sumk], i32)
            nc.vector.tensor_copy(s32[:], s16[:])
            nc.vector.tensor_scalar(out=s32[:], in0=s32[:], scalar1=IDX_OFF,
                                    scalar2=None, op0=ALU.add)

            erA_sb = cpool.tile([PART, BLOCKS, 8], f16)
            erB_sb = cpool.tile([PART, BLOCKS, 2], f16)
            erC_sb = cpool.tile([PART, BLOCKS, 2], f16)

            # sentinel rows (feat 0, el slots EL_SENT)
            sentA = cpool.tile([1, A_ROW], f16)
            nc.vector.memset(sentA[:], 0.0)
            nc.vector.memset(sentA[:, 256:260], EL_SENT)
            nc.vector.memset(sentA[:, 516:520], EL_SENT)
            nc.sync.dma_start(tableA[S_TOT:S_TOT + 1, :], sentA[:])
            sentB = cpool.tile([1, B_ROW], f16)
            nc.vector.memset(sentB[:], 0.0)
            nc.vector.memset(sentB[:, 64:65], EL_SENT)
            nc.vector.memset(sentB[:, 129:130], EL_SENT)
            nc.sync.dma_start(tableB[S_TOT:S_TOT + 1, :], sentB[:])
            sentC = cpool.tile([1, C_ROW], f16)
            nc.vector.memset(sentC[:], 0.0)
            nc.vector.memset(sentC[:, 40:41], EL_SENT)
            nc.vector.memset(sentC[:, 81:82], EL_SENT)
            nc.sync.dma_start(tableC[S_TOT:S_TOT + 1, :], sentC[:])

            # ---- phase 1: GEMM-A (node-sharded) + all-gather ----
            # x arrives 12-bit packed (node pairs in 3 bytes); unpack to
            # integer-valued f16, matmul, then fold the per-node scale into
            # the PSUM->row copy (scale factors out of the GEMM row).
            with tc.tile_pool(name="psA", bufs=2, space="PSUM") as gpp:
                for b in range(BLOCKS):
                    sl = slice(b * PART, (b + 1) * PART)
                    x8 = gip.tile([PART, 2, 192], u8, tag="x8")
                    nc.sync.dma_start(x8[:],
                                      xpk[:, :, 192 * b:192 * (b + 1)])
                    lo = gip.tile([PART, 2, 64], i16, tag="xlo")
                    nc.vector.tensor_copy(lo[:], x8[:, :, 0::3])
                    mid = gip.tile([PART, 2, 64], i16, tag="xmid")
                    nc.vector.tensor_copy(mid[:], x8[:, :, 1::3])
                    hi = gip.tile([PART, 2, 64], i16, tag="xhi")
                    nc.vector.tensor_copy(hi[:], x8[:, :, 2::3])
                    t1 = gip.tile([PART, 2, 64], i16, tag="xt1")
                    nc.vector.tensor_scalar(out=t1[:], in0=mid[:], scalar1=15,
                                            scalar2=None, op0=ALU.bitwise_and)
                    nc.vector.tensor_scalar(out=t1[:], in0=t1[:], scalar1=8,
                                            scalar2=None,
                                            op0=ALU.logical_shift_left)
                    nc.vector.tensor_tensor(out=lo[:], in0=lo[:], in1=t1[:],
                                            op=ALU.add)
                    nc.vector.tensor_scalar(out=mid[:], in0=mid[:], scalar1=4,
                                            scalar2=None,
                                            op0=ALU.logical_shift_right)
                    nc.vector.tensor_scalar(out=hi[:], in0=hi[:], scalar1=4,
                                            scalar2=None,
                                            op0=ALU.logical_shift_left)
                    nc.vector.tensor_tensor(out=mid[:], in0=mid[:], in1=hi[:],
                                            op=ALU.add)
                    xt = gip.tile([PART, 2, PART], f16, tag="xt")
                    nc.vector.tensor_scalar(out=xt[:, :, 0::2], in0=lo[:],
                                            scalar1=-2048, scalar2=None,
                                            op0=ALU.add)
                    nc.vector.tensor_scalar(out=xt[:, :, 1::2], in0=mid[:],
                                            scalar1=-2048, scalar2=None,
                                            op0=ALU.add)
                    ps = gpp.tile([PART, A_W], f32, space="PSUM", tag="psA")
                    for k2 in range(2):
                        nc.tensor.matmul(ps[:, 0:512], lhsT=xt[:, k2, :],
                                         rhs=wa_t[:, k2, 0:512],
                                         start=(k2 == 0), stop=(k2 == 1))
                        nc.tensor.matmul(ps[:, 512:A_W], lhsT=xt[:, k2, :],
                                         rhs=wa_t[:, k2, 512:A_W],
                                         start=(k2 == 0), stop=(k2 == 1))
                    row = gop.tile([PART, A_ROW], f16, tag="rowA")
                    nc.vector.tensor_tensor(
                        out=row[:], in0=ps[:, 0:A_ROW],
                        in1=sx_t[:, b:b + 1].to_broadcast([PART, A_ROW]),
                        op=ALU.mult)
                    nc.vector.tensor_tensor(
                        out=erA_sb[:, b, :], in0=ps[:, A_ROW:A_W],
                        in1=sx_t[:, b:b + 1].to_broadcast([PART, 8]),
                        op=ALU.mult)
                    nc.sync.dma_start(tbA_sh[sl, :], row[:])
            nc.gpsimd.collective_compute(
                "AllGather", ALU.bypass, replica_groups=groups,
                ins=[tbA_sh[:, :]], outs=[tableA[0:S_TOT, :]])

            # ---- edge phase helper ----
            def edge_phase(table, row_w, er_sb, fdim, nheads, out_cb):
                """One GAT aggregation layer over all blocks (both branches).

                row layout per branch: [feat fdim*nheads | el nheads]
                """
                fw = fdim * nheads
                r = fw + nheads
                for b in range(BLOCKS):
                    kb = K[b]
                    g = gap.tile([PART, kb, row_w], f16, tag="g")
                    for k in range(kb):
                        c0 = int(off[b]) + k
                        nc.gpsimd.indirect_dma_start(
                            out=g[:, k, :], out_offset=None,
                            in_=table[:, :],
                            in_offset=bass.IndirectOffsetOnAxis(
                                ap=s32[:, c0:c0 + 1], axis=0))
                    # e = el + er  [128, kb, 2, H] f32
                    el = g[:].rearrange("p k (b2 r) -> p k b2 r", b2=2)[
                        :, :, :, fw:fw + nheads]
                    e = smp.tile([PART, kb, 2, nheads], f32, tag="e")
                    erv = er_sb[:, b, :].rearrange(
                        "p (o b2 h) -> p o b2 h", o=1, b2=2)
                    nc.vector.tensor_tensor(
                        out=e[:], in0=el,
                        in1=erv.to_broadcast([PART, kb, 2, nheads]),
                        op=ALU.add)
                    # m = lrelu(max_k e); e = lrelu(e) - m
                    # (ACT.Lrelu ignores alpha and uses slope 0.01, so leaky
                    # relu is computed manually: max(x, 0.2*x))
                    m = smp.tile([PART, 1, 2, nheads], f32, tag="m")
                    nc.vector.tensor_reduce(
                        out=m[:], in_=e[:].rearrange("p k b2 h -> p b2 h k"),
                        axis=X, op=ALU.max)
                    m2 = smp.tile([PART, 1, 2, nheads], f32, tag="m2")
                    nc.vector.tensor_scalar(out=m2[:], in0=m[:], scalar1=0.2,
                                            scalar2=None, op0=ALU.mult)
                    nc.vector.tensor_tensor(out=m[:], in0=m[:], in1=m2[:],
                                            op=ALU.max)
                    e2 = smp.tile([PART, kb, 2, nheads], f32, tag="e2")
                    nc.vector.tensor_scalar(out=e2[:], in0=e[:], scalar1=0.2,
                                            scalar2=None, op0=ALU.mult)
                    nc.vector.tensor_tensor(out=e[:], in0=e[:], in1=e2[:],
                                            op=ALU.max)
                    nc.vector.tensor_tensor(
                        out=e[:], in0=e[:],
                        in1=m[:].to_broadcast([PART, kb, 2, nheads]),
                        op=ALU.subtract)
                    ex = smp.tile([PART, kb, 2, nheads], f16, tag="ex")
                    nc.scalar.activation(ex[:], e[:], ACT.Exp)
                    # den / reciprocal
                    den = smp.tile([PART, 1, 2, nheads], f32, tag="den")
                    nc.vector.tensor_reduce(
                        out=den[:], in_=ex[:].rearrange("p k b2 h -> p b2 h k"),
                        axis=X, op=ALU.add)
                    nc.vector.tensor_scalar(out=den[:], in0=den[:],
                                            scalar1=1e-9, scalar2=None,
                                            op0=ALU.max)
                    rec = smp.tile([PART, 1, 2, nheads], f32, tag="rec")
                    nc.vector.reciprocal(rec[:], den[:])
                    # g(feat) *= ex
                    gf = g[:].rearrange("p k (b2 r) -> p k b2 r", b2=2)[
                        :, :, :, 0:fw].rearrange(
                        "p k b2 (h d) -> p k b2 h d", h=nheads)
                    exb = ex[:].rearrange(
                        "p k b2 (h o) -> p k b2 h o", o=1).to_broadcast(
                        [PART, kb, 2, nheads, fdim])
                    nc.vector.tensor_tensor(out=gf, in0=gf, in1=exb,
                                            op=ALU.mult)
                    # msum over k -> [128, 2, H, fdim] f32; rst = msum * rec
                    ms = epp.tile([PART, 2, nheads, fdim], f32, tag="ms")
                    gfk = g[:].rearrange("p k (b2 r) -> p k b2 r", b2=2)[
                        :, :, :, 0:fw].rearrange(
                        "p k b2 (h d) -> p b2 h d k", h=nheads)
                    nc.vector.tensor_reduce(out=ms[:], in_=gfk, axis=X,
                                            op=ALU.add)
                    o = epp.tile([PART, 2 * fw], f32, tag="o")
                    ov = o[:].rearrange("p (b2 h d) -> p b2 h d", b2=2,
                                        h=nheads)
                    recb = rec[:].rearrange(
                        "p o b2 (h o2) -> p o b2 h o2", o2=1)[:, 0]
                    nc.vector.tensor_tensor(
                        out=ov, in0=ms[:],
                        in1=recb.to_broadcast([PART, 2, nheads, fdim]),
                        op=ALU.mult)
                    out_cb(b, o)

            def elu_inplace(ap, width):
                """ap <- elu(ap): relu(x) + exp(min(x,0)) - 1."""
                tm = epp.tile([PART, width], f32, tag="elu_t")
                nc.vector.tensor_scalar(out=tm[:], in0=ap, scalar1=0.0,
                                        scalar2=None, op0=ALU.min)
                te = epp.tile([PART, width], f32, tag="elu_e")
                nc.scalar.activation(te[:], tm[:], ACT.Exp)
                nc.scalar.activation(tm[:], ap, ACT.Relu)
                nc.vector.tensor_tensor(out=te[:], in0=te[:], in1=tm[:],
                                        op=ALU.add)
                nc.vector.tensor_scalar(out=ap, in0=te[:], scalar1=-1.0,
                                        scalar2=None, op0=ALU.add)

            # ---- edge phase A -> aggA ----
            def out_a(b, o):
                elu_inplace(o[:], 512)
                o16 = epp.tile([PART, 512], f16, tag="o16")
                nc.scalar.copy(o16[:], o[:])
                nc.sync.dma_start(aggA[b * PART:(b + 1) * PART, :], o16[:])

            edge_phase(tableA, A_ROW, erA_sb, 64, 4, out_a)

            # ---- phase 3: GEMM-B (sharded) + all-gather ----
            with tc.tile_pool(name="psB", bufs=2, space="PSUM") as gpp:
                for b in range(BLOCKS):
                    sl = slice(b * PART, (b + 1) * PART)
                    hT = gip.tile([PART, 4, PART], f16, tag="hT")
                    for k2 in range(4):
                        nc.sync.dma_start_transpose(
                            hT[:, k2, :],
                            aggA[sl, k2 * PART:(k2 + 1) * PART])
                    psb = gpp.tile([PART, B_W], f32, space="PSUM", tag="psB")
                    for k2 in range(4):
                        nc.tensor.matmul(psb[:], lhsT=hT[:, k2, :],
                                         rhs=wb_t[:, k2, :],
                                         start=(k2 == 0), stop=(k2 == 3))
                    rowb = gop.tile([PART, B_ROW], f16, tag="rowB")
                    nc.vector.tensor_copy(rowb[:], psb[:, 0:B_ROW])
                    nc.vector.tensor_copy(erB_sb[:, b, :], psb[:, 130:132])
                    nc.sync.dma_start(tbB_sh[sl, :], rowb[:])
            nc.gpsimd.collective_compute(
                "AllGather", ALU.bypass, replica_groups=groups,
                ins=[tbB_sh[:, :]], outs=[tableB[0:S_TOT, :]])

            # ---- edge phase B -> aggB ----
            def out_b(b, o):
                elu_inplace(o[:, 0:64], 64)
                o16 = epp.tile([PART, PART], f16, tag="o16b")
                nc.scalar.copy(o16[:], o[:])
                nc.sync.dma_start(aggB[b * PART:(b + 1) * PART, :], o16[:])

            edge_phase(tableB, B_ROW, erB_sb, 64, 1, out_b)

            # ---- phase 5: GEMM-C (sharded) + all-gather ----
            with tc.tile_pool(name="psC", bufs=2, space="PSUM") as gpp:
                for b in range(BLOCKS):
                    sl = slice(b * PART, (b + 1) * PART)
                    hc = gip.tile([PART, PART], f16, tag="hc")
                    nc.sync.dma_start_transpose(hc[:], aggB[sl, :])
                    psc = gpp.tile([PART, C_W], f32, space="PSUM", tag="psC")
                    nc.tensor.matmul(psc[:], lhsT=hc[:], rhs=wc_t[:, 0, :],
                                     start=True, stop=True)
                    rowc = gop.tile([PART, C_ROW], f16, tag="rowC")
                    nc.vector.tensor_copy(rowc[:], psc[:, 0:C_ROW])
                    nc.vector.tensor_copy(erC_sb[:, b, :], psc[:, 82:84])
                    nc.sync.dma_start(tbC_sh[sl, :], rowc[:])
            nc.gpsimd.collective_compute(
                "AllGather", ALU.bypass, replica_groups=groups,
                ins=[tbC_sh[:, :]], outs=[tableC[0:S_TOT, :]])

            # ---- edge phase C -> y ----
            def out_c(b, o):
                elu_inplace(o[:, 40:80], 40)
                # int8 output: y in [-3.37, 3.37], quant step 1/38 (rounds+
                # saturates on conversion); host divides by Y_SCALE
                oi8 = epp.tile([PART, 2 * C_OUT], i8, tag="oi8")
                nc.vector.tensor_scalar(out=oi8[:], in0=o[:], scalar1=Y_SCALE,
                                        scalar2=None, op0=ALU.mult)
                nc.sync.dma_start(y_d[b * PART:(b + 1) * PART, :], oi8[:])

            edge_phase(tableC, C_ROW, erC_sb, 40, 1, out_c)

    nc.compile()
    return nc


# ----------------------------------------------------------------------------
# entry point
# ----------------------------------------------------------------------------
def _get_compiled(K):
    if K not in _COMPILED:
        _COMPILED[K] = _build(K)
    return _COMPILED[K]


def kernel(**inputs):
    src = np.asarray(inputs["src"]).astype(np.int64) % N_NODES
    dst = np.asarray(inputs["dst"]).astype(np.int64) % N_NODES

    sched = _schedule(src, dst)
    in_maps = _prep_inputs(inputs, sched)
    nc = _get_compiled(sched["K"])

    from concourse.bass_utils import run_bass_kernel_spmd
    res = run_bass_kernel_spmd(nc, in_maps, list(range(N_CORES)))

    y0 = np.zeros((N_NODES, C_OUT), np.float32)
    y1 = np.zeros((N_NODES, C_OUT), np.float32)
    ncore = sched["node_core"]
    sic = sched["slot_in_core"]
    for c in range(N_CORES):
        nodes = np.where(ncore == c)[0]
        yc = res.results[c]["y"].astype(np.float32) / Y_SCALE
        y0[nodes] = yc[sic[nodes], 0:C_OUT]
        y1[nodes] = yc[sic[nodes], C_OUT:2 * C_OUT]
    out = np.stack([y0, y1], axis=0)
    _STATE["last"] = (nc, in_maps, sched)
    return out
